# revision 55
# baseline (speedup 1.0000x reference)
"""Trainium2 Bass kernel for nn_Model_14336600834737 (dense_transformer).

Model (per batch element b, N=2048 tokens, D=384):
  x = LN(sincos_embed(q) + group_emb)
  8 blocks:
    h = LN_l(x);  z = h @ We^T  (f = [qh 48 | kh 48 | lin 768 | pre_gelu 768])
    geglu = lin * gelu(pre_gelu);  local, attn_v = geglu[:, :384], geglu[:, 384:]
    scores = qh kh^T / sqrt(48) + mask_l   (mask = sigmoid(j-i+pbm) causal, -inf else)
    att = softmax(scores) @ attn_v
    x = x + [local | att] @ P^T
  out = LN(x) @ out_w^T   -> [2048, 100]

Sharding: pure data parallel, 1 batch element per NeuronCore (8 cores).
On-chip layout: residual kept transposed xT [d, n] = 3 chunks of [128, 2048];
attention computed as S^T [j, i] tiles so softmax sums use ones-matmuls and
attention output lands directly as att^T [d, i] (no transposes anywhere).

v2: bf16 activations + weights (psum/residual stay fp32); all per-layer
tensors double-buffered so layer l+1 expand overlaps layer l attention;
ACT queue chained in emission order with only two table sets
(natural_log_exp for LN-rstd/mask/softmax, gelu set for geglu) so table
reloads drop from ~180 to ~18; qh/kh computed in one matmul.
"""

import math

import numpy as np

# ---- hardcoded problem shapes ----
B, N, D = 8, 2048, 384
NB, QK, EX, GS, VOCAB = 8, 48, 768, 2, 100
NT = 4            # n tiles per 2048
TW = 512          # tile width
NBLK = N // 128   # 16 row blocks
NC = 3            # d chunks (384/128)
NE = 6            # e chunks (768/128)
MLW = 1152        # mask window width
MLA = 384         # mask window offset: gwin[p,u] = p - u + MLA
INV_SQRT_QK = 1.0 / math.sqrt(QK)
QKP = 112         # padded qk rows: [qh 0:48 | 0 pad 48:64 | kh 64:112]
KOFF = 64         # kh partition offset (32-aligned)
QK4 = INV_SQRT_QK ** 0.5   # scale applied to both qh and kh
NEG_BIG = -1.0e30
N_CORES = 8

_CACHE = {}


def _install_ntff_hook():
    """Register the axon NTFF profile hook if missing (enables trace=True)."""
    import sys, types
    if "antenv.axon_hooks" in sys.modules:
        return
    mod = types.ModuleType("antenv.axon_hooks")
    _h = [None]
    mod.set_axon_ntff_profile_hook = lambda h: _h.__setitem__(0, h)
    mod.get_axon_ntff_profile_hook = lambda: _h[0]
    sys.modules["antenv.axon_hooks"] = mod
    try:
        import antenv
        antenv.axon_hooks = mod
        from trn_agent_boot.trn_boot import _ntff_profile_via_ctypes
        hook = _ntff_profile_via_ctypes("/opt/axon/libaxon_pjrt.so")
        if hook is not None:
            mod.set_axon_ntff_profile_hook(hook)
    except Exception:
        pass


def build_nc(trivial_ln=False, trivial_norm=False):
    import concourse.bacc as bacc
    import concourse.tile as tile
    from concourse.tile import add_dep_helper
    from concourse import mybir

    dt = mybir.dt
    AF = mybir.ActivationFunctionType
    ALU = mybir.AluOpType
    F32, F32R, BF16 = dt.float32, dt.float32r, dt.bfloat16
    FP8 = dt.float8e4
    DR = mybir.MatmulPerfMode.DoubleRow

    nc = bacc.Bacc("TRN2", target_bir_lowering=False, debug=False)

    # ---- DRAM I/O ----
    d_q = nc.dram_tensor("qrow", [1, N], F32, kind="ExternalInput")
    d_fvec = nc.dram_tensor("fvec", [128, NC], F32, kind="ExternalInput")
    d_bvec = nc.dram_tensor("bvec", [128, NC], F32, kind="ExternalInput")
    d_ge = nc.dram_tensor("ge", [128, NC, GS], F32, kind="ExternalInput")
    d_lng = nc.dram_tensor("lng", [NB, 128, NC], F32, kind="ExternalInput")
    d_lnb = nc.dram_tensor("lnb", [NB, 128, NC], F32, kind="ExternalInput")
    d_ng = nc.dram_tensor("nrmg", [128, NC], F32, kind="ExternalInput")
    d_nb = nc.dram_tensor("nrmb", [128, NC], F32, kind="ExternalInput")
    d_mask = nc.dram_tensor("mask8", [NB, 128, MLW], F32, kind="ExternalInput")
    d_eA = nc.dram_tensor("eA", [NB, NC, 128, 880], BF16, kind="ExternalInput")
    d_eB = nc.dram_tensor("eB", [NB, NC, 128, 2 * D], BF16, kind="ExternalInput")
    d_pW = nc.dram_tensor("pW", [NB, NE // 2, 128, 2, D], FP8,
                          kind="ExternalInput")
    d_oW = nc.dram_tensor("oW", [NC, 128, VOCAB], BF16, kind="ExternalInput")
    d_out = nc.dram_tensor("logits", [N, VOCAB], F32, kind="ExternalOutput")

    def mm(out, lhsT, rhs, start, stop):
        """fp32 matmul in f32r fast mode."""
        nc.tensor.matmul(out, lhsT.bitcast(F32R), rhs.bitcast(F32R),
                         start=start, stop=stop)

    def mmb(out, lhsT, rhs, start, stop):
        """bf16 matmul."""
        nc.tensor.matmul(out, lhsT, rhs, start=start, stop=stop)

    with tile.TileContext(nc) as tc:
        import contextlib
        with contextlib.ExitStack() as ctx:
            ctx.enter_context(nc.allow_low_precision(
                reason="bf16/f32r matmuls within tolerance"))
            sb = ctx.enter_context(tc.tile_pool(name="sb", bufs=1))
            ps = ctx.enter_context(tc.tile_pool(name="ps", bufs=1, space="PSUM"))

            # ACT-queue chain: keep scalar-engine ops in emission order so the
            # scheduler can't interleave gelu-set and exp-set ops (each
            # alternation costs a ~1.3-2.7us table reload).
            _act_prev = [None]

            def _chain(i):
                if _act_prev[0] is not None:
                    add_dep_helper(i.ins, _act_prev[0].ins, sync=False,
                                   reason="ACT table-set phase order")
                _act_prev[0] = i
                return i

            def act(**kw):
                return _chain(nc.scalar.activation(**kw))

            def actc(out, in_):
                return _chain(nc.scalar.copy(out=out, in_=in_))

            # ---- constants into SBUF ----
            fvec = sb.tile([128, NC], F32, tag="fvec")
            nc.sync.dma_start(out=fvec, in_=d_fvec.ap())
            bvec = sb.tile([128, NC], F32, tag="bvec")
            nc.sync.dma_start(out=bvec, in_=d_bvec.ap())
            ge = sb.tile([128, NC, GS], F32, tag="ge")
            nc.sync.dma_start(out=ge, in_=d_ge.ap())
            lng = sb.tile([128, NB * NC], F32, tag="lng")
            lnb = sb.tile([128, NB * NC], F32, tag="lnb")
            for l in range(NB):
                nc.sync.dma_start(out=lng[:, NC * l:NC * l + NC], in_=d_lng.ap()[l])
                nc.sync.dma_start(out=lnb[:, NC * l:NC * l + NC], in_=d_lnb.ap()[l])
            nrmg = sb.tile([128, NC], F32, tag="nrmg")
            nc.sync.dma_start(out=nrmg, in_=d_ng.ap())
            nrmb = sb.tile([128, NC], F32, tag="nrmb")
            nc.sync.dma_start(out=nrmb, in_=d_nb.ap())
            oW = sb.tile([128, NC, VOCAB], BF16, tag="oW")
            for c in range(NC):
                nc.sync.dma_start(out=oW[:, c, :], in_=d_oW.ap()[c])
            qsb = sb.tile([128, N], F32, tag="qsb")
            nc.sync.dma_start(out=qsb, in_=d_q.ap().broadcast_to((128, N)))

            # ones1 holds 1/D so the LN stats matmuls produce means directly
            ones1 = sb.tile([128, 1], F32R, tag="ones1")
            nc.vector.memset(ones1.bitcast(F32), 1.0 / D)
            ones2 = sb.tile([128, 2, 16], FP8, tag="ones2")
            nc.vector.memset(ones2, 1.0)
            onesr = sb.tile([1, 128], F32R, tag="onesr")
            nc.scalar.activation(out=onesr, in_=onesr.bitcast(F32),
                                 func=AF.Identity, scale=0.0, bias=1.0)
            eps = sb.tile([1, 1], F32, tag="eps")
            nc.vector.memset(eps, 1e-5)

            # ---- persistent activations ----
            xT = [sb.tile([128, N], F32, tag=f"x{c}", name=f"xT{c}") for c in range(NC)]

            def bcast_row(row_ap):
                """[1, TW] -> PSUM [128, TW] via K=1 ones matmul."""
                out = ps.tile([128, TW], F32, tag="att", bufs=3, name="bc")
                mm(out, onesr, row_ap, True, True)
                return out

            def layer_norm(dst_of, g_ap, b_ap, trivial_affine=False,
                           dst_dt=BF16):
                """LN over d (partition chunks) of xT -> dst tiles per (c, nt).

                Stats via ones-matmuls; rstd = exp(-0.5*ln(var+eps)) so the
                scalar engine never leaves the natural_log_exp table set;
                broadcasts via K=1 matmuls.
                """
                mus, sds = {}, {}
                for nt in range(NT):
                    s1 = ps.tile([1, TW], F32, tag="row", bufs=1, name="s1")
                    sqs = []
                    for c in range(NC):
                        xs = xT[c][:, TW * nt:TW * nt + TW]
                        sq = sb.tile([128, TW], F32, tag="sq", bufs=3, name="sq")
                        nc.gpsimd.tensor_mul(sq.bitcast(F32R), xs, xs)
                        sqs.append(sq)
                        mm(s1, ones1, xs, c == 0, c == NC - 1)
                    mu = sb.tile([1, TW], F32, tag="mu", bufs=NT, name="mu")
                    nc.vector.tensor_copy(mu.bitcast(F32R), s1)  # = mean
                    s2 = ps.tile([1, TW], F32, tag="row", bufs=1, name="s2")
                    for c in range(NC):
                        mm(s2, ones1, sqs[c], c == 0, c == NC - 1)
                    m2 = sb.tile([1, TW], F32, tag="m2", bufs=1, name="m2")
                    nc.vector.tensor_mul(m2, mu, mu)
                    msq = sb.tile([1, TW], F32, tag="msq", bufs=NT, name="msq")
                    nc.vector.tensor_sub(msq.bitcast(F32R), s2, m2)  # var
                    mus[nt], sds[nt] = mu, msq
                for nt in range(NT):
                    # sd = sqrt(var + eps); all 4 chained -> one table set
                    act(out=sds[nt].bitcast(F32R), in_=sds[nt], func=AF.Sqrt,
                        bias=eps)
                for nt in range(NT):
                    mu_b = bcast_row(mus[nt])
                    sd_b = bcast_row(sds[nt])
                    rstdb = sb.tile([128, TW], F32, tag="rstdb", bufs=2,
                                    name="rstdb")
                    nc.vector.reciprocal_approx_fast(out=rstdb, in_=sd_b)
                    for c in range(NC):
                        xs = xT[c][:, TW * nt:TW * nt + TW]
                        t1 = sb.tile([128, TW], F32, tag="t1", bufs=2, name="t1")
                        nc.vector.tensor_sub(t1, xs, mu_b)
                        if trivial_affine:
                            nc.vector.tensor_mul(dst_of(c, nt, dst_dt), t1,
                                                 rstdb)
                        else:
                            nc.vector.tensor_mul(t1, t1, rstdb)
                            nc.scalar.activation(out=dst_of(c, nt, dst_dt),
                                                 in_=t1, func=AF.Identity,
                                                 bias=b_ap[:, c:c + 1],
                                                 scale=g_ap[:, c:c + 1])

            # ---- embedding: xT[c][p, n] = sin(q[n]*fvec[p,c] + bvec[p,c]) ----
            for c in range(NC):
                act(out=xT[c].bitcast(F32R), in_=qsb, func=AF.Sin,
                    scale=fvec[:, c:c + 1], bias=bvec[:, c:c + 1])
                xr = xT[c].rearrange("p (a g) -> p a g", g=GS)
                geb = ge[:, c, :].unsqueeze(1).broadcast_to((128, N // GS, GS))
                nc.vector.tensor_add(xr.bitcast(F32R), xr, geb)

            # initial LN (in place on xT, fp32)
            def init_dst(c, nt, _dt):
                return xT[c][:, TW * nt:TW * nt + TW].bitcast(F32R)

            layer_norm(init_dst, nrmg, nrmb, trivial_affine=trivial_norm,
                       dst_dt=F32)

            # ---- transformer layers ----
            for l in range(NB):
                # layer weights (double-buffered across layers)
                eA = [sb.tile([128, 880], BF16, tag=f"eA{c}", bufs=2,
                              name=f"eA{c}_{l}") for c in range(NC)]
                eB = [sb.tile([128, 2 * D], BF16, tag=f"eB{c}", bufs=2,
                              name=f"eB{c}_{l}") for c in range(NC)]
                for c in range(NC):
                    nc.sync.dma_start(out=eA[c], in_=d_eA.ap()[l, c])
                    nc.sync.dma_start(out=eB[c], in_=d_eB.ap()[l, c])
                pW2 = [sb.tile([128, 2, D], FP8, tag=f"pW{k}", bufs=2,
                               name=f"pW{k}_{l}") for k in range(NE // 2)]
                for k in range(NE // 2):
                    nc.sync.dma_start(out=pW2[k], in_=d_pW.ap()[l, k])

                # per-layer additive mask window, precomputed on host:
                # sigmoid(j - i + pbm_l) where causal, -1e30 where not.
                Ml = sb.tile([128, MLW], F32, tag="Ml", bufs=2, name=f"Ml_{l}")
                nc.sync.dma_start(out=Ml, in_=d_mask.ap()[l])

                # h = LN_l(x), then expand + geglu, per n-tile
                qh = sb.tile([QK, N], BF16, tag="qh", bufs=2, name=f"qh_{l}")
                kh = sb.tile([QK, N], BF16, tag="kh", bufs=2, name=f"kh_{l}")
                # projection rhs packed in fp8 e-chunk pairs for DoubleRow:
                # prj[k][p, o, n]; pairs = (loc0,loc1), (loc2,att0), (att1,att2)
                prj = [sb.tile([128, 2, N], FP8, tag=f"prj{k}", bufs=2,
                               name=f"prj{k}_{l}") for k in range(NE // 2)]
                # V packed in fp8 j-block pairs for DoubleRow P@V:
                # V2[jp][p, o, d] = V[j = 128*(2jp+o) + p][d]
                V2 = [sb.tile([128, 2, D], FP8, tag="v", bufs=NBLK,
                              name=f"v{jp}_{l}") for jp in range(NBLK // 2)]

                hcur = {}

                def make_h(c, nt, _dt, _l=l):
                    hcur[(c, nt)] = sb.tile([128, TW], _dt, tag="h", bufs=6,
                                            name=f"h{c}_{nt}_{_l}")
                    return hcur[(c, nt)]

                layer_norm(make_h, lng[:, NC * l:NC * l + NC],
                           lnb[:, NC * l:NC * l + NC], trivial_affine=trivial_ln)

                for nt in range(NT):
                    hnt = [hcur[(c, nt)] for c in range(NC)]
                    # qh/kh (transposed layout); 1/sqrt(QK) folded into the
                    # qh weight columns host-side; copies on the scalar engine
                    pq = ps.tile([QK, TW], F32, tag="s", bufs=4, name="pq")
                    for c in range(NC):
                        mmb(pq, eA[c][:, 0:QK], hnt[c], c == 0, c == NC - 1)
                    actc(qh[:, TW * nt:TW * nt + TW], pq)
                    pk = ps.tile([QK, TW], F32, tag="s", bufs=4, name="pk")
                    for c in range(NC):
                        mmb(pk, eA[c][:, KOFF:KOFF + QK], hnt[c],
                            c == 0, c == NC - 1)
                    actc(kh[:, TW * nt:TW * nt + TW], pk)
                    # local geglu, transposed layout [e-block, n]
                    for e in range(NC):
                        pl = ps.tile([128, TW], F32, tag="s", bufs=4, name="pl")
                        pg = ps.tile([128, TW], F32, tag="s", bufs=4, name="pg")
                        for c in range(NC):
                            mmb(pl, eA[c][:, QKP + 128 * e:QKP + 128 + 128 * e],
                                hnt[c], c == 0, c == NC - 1)
                        for c in range(NC):
                            mmb(pg, eA[c][:, 496 + 128 * e:624 + 128 * e],
                                hnt[c], c == 0, c == NC - 1)
                        gt = sb.tile([128, TW], F32, tag="gt", bufs=3, name="gt")
                        act(out=gt, in_=pg, func=AF.Gelu)
                        nc.vector.tensor_mul(
                            prj[e // 2][:, e % 2, TW * nt:TW * nt + TW], pl, gt)
                    # attn_v geglu, row layout V[j, d]
                    for k in range(NT):
                        nb = NT * nt + k
                        pvl = ps.tile([128, D], F32, tag="s", bufs=4, name="pvl")
                        pvg = ps.tile([128, D], F32, tag="s", bufs=4, name="pvg")
                        for c in range(NC):
                            mmb(pvl, hnt[c][:, 128 * k:128 * k + 128],
                                eB[c][:, 0:D], c == 0, c == NC - 1)
                        for c in range(NC):
                            mmb(pvg, hnt[c][:, 128 * k:128 * k + 128],
                                eB[c][:, D:2 * D], c == 0, c == NC - 1)
                        gt2 = sb.tile([128, D], F32, tag="gt", bufs=3, name="gt2")
                        act(out=gt2, in_=pvg, func=AF.Gelu)
                        nc.vector.tensor_mul(V2[nb // 2][:, nb % 2, :], pvl, gt2)

                # ---- attention, i-chunks of 512, S^T [j, i] tiles ----
                for ic in range(NT):
                    i0 = TW * ic
                    nj = NT * (ic + 1)
                    pa = [ps.tile([128, TW], F32, tag="att", bufs=3, name=f"pa{d}")
                          for d in range(NC)]
                    prow = ps.tile([1, TW], F32, tag="row", bufs=1, name="prow")
                    njp = nj // 2
                    for jp in range(njp):
                        es2 = sb.tile([128, 2, TW], FP8, tag="es", bufs=4,
                                      name="es2")
                        for o in range(2):
                            jb = 2 * jp + o
                            j0 = 128 * jb
                            # mask needed only near the diagonal: sigmoid
                            # underflows below offset -128
                            need = j0 - i0 >= -128
                            pss = ps.tile([128, TW], F32, tag="s", bufs=4,
                                          name="pss")
                            mmb(pss, kh[:, j0:j0 + 128], qh[:, i0:i0 + TW],
                                True, True)
                            if not need:
                                act(out=es2[:, o, :], in_=pss, func=AF.Exp)
                            else:
                                off = MLA - (j0 - i0)
                                em = sb.tile([128, TW], F32, tag="em", bufs=2,
                                             name="em")
                                nc.vector.tensor_add(em, pss,
                                                     Ml[:, off:off + TW])
                                act(out=es2[:, o, :], in_=em, func=AF.Exp)
                        nc.tensor.matmul(prow, ones2[:, :, 0:1], es2,
                                         start=jp == 0, stop=jp == njp - 1,
                                         perf_mode=DR)
                        for d in range(NC):
                            nc.tensor.matmul(
                                pa[d], V2[jp][:, :, 128 * d:128 * d + 128],
                                es2, start=jp == 0, stop=jp == njp - 1,
                                perf_mode=DR)
                    rs = sb.tile([1, TW], F32, tag="rs", bufs=2, name="rs")
                    nc.vector.tensor_copy(rs.bitcast(F32R), prow)
                    s_b = ps.tile([128, TW], F32, tag="row", bufs=1, name="s_b")
                    mm(s_b, onesr, rs, True, True)
                    rsb = sb.tile([128, TW], F32, tag="rsb", bufs=2, name="rsb")
                    nc.vector.reciprocal_approx_fast(out=rsb, in_=s_b)
                    # att d-chunk d lands in prj pair (NC+d)//2, slot (NC+d)%2
                    for d in range(NC):
                        e = NC + d
                        nc.vector.tensor_mul(
                            prj[e // 2][:, e % 2, i0:i0 + TW], pa[d], rsb)
                    # projection for this n-tile + residual (fp8 DoubleRow)
                    for d2 in range(NC):
                        pp = ps.tile([128, TW], F32, tag="s", bufs=4, name="pp")
                        for k in range(NE // 2):
                            nc.tensor.matmul(
                                pp, pW2[k][:, :, 128 * d2:128 * d2 + 128],
                                prj[k][:, :, i0:i0 + TW],
                                start=k == 0, stop=k == NE // 2 - 1,
                                perf_mode=DR)
                        xs = xT[d2][:, i0:i0 + TW]
                        nc.vector.tensor_add(xs.bitcast(F32R), pp, xs)

            # ---- final LN + head ----
            hf = {}

            def make_hf(c, nt, _dt):
                hf[(c, nt)] = sb.tile([128, TW], _dt, tag="h", bufs=6,
                                      name=f"hf{c}_{nt}")
                return hf[(c, nt)]

            layer_norm(make_hf, nrmg, nrmb, trivial_affine=trivial_norm)
            for nt in range(NT):
                for k in range(NT):
                    nb = NT * nt + k
                    po = ps.tile([128, VOCAB], F32, tag="s", bufs=4, name="po")
                    for c in range(NC):
                        mmb(po, hf[(c, nt)][:, 128 * k:128 * k + 128],
                            oW[:, c, :], c == 0, c == NC - 1)
                    ot = sb.tile([128, VOCAB], F32, tag="ot", bufs=2, name="ot")
                    nc.scalar.copy(out=ot, in_=po)
                    nc.sync.dma_start(out=d_out.ap()[128 * nb:128 * nb + 128, :],
                                      in_=ot)

    nc.compile()
    return nc


def _prep_inputs(q, group_emb, expand_w, project_w, ln_g, ln_b, pbm, norm_g,
                 norm_b, out_w):
    """Host-side sharding + weight layout prep. Returns list of per-core maps."""
    import ml_dtypes
    f32 = np.float32
    bf16 = ml_dtypes.bfloat16
    freqs = np.exp(np.arange(0, D, 2, dtype=np.float64)
                   * (-math.log(10000.0) / D)).astype(f32)  # [192]
    fv = np.zeros((D,), f32)
    bv = np.zeros((D,), f32)
    fv[:192] = freqs
    fv[192:] = freqs
    bv[192:] = math.pi / 2.0
    fvec = fv.reshape(NC, 128).T.copy()           # [128, 3]
    bvec = bv.reshape(NC, 128).T.copy()

    ge = np.asarray(group_emb, f32).reshape(GS, D)      # [2, 384]
    geT = np.transpose(ge.reshape(GS, NC, 128), (2, 1, 0)).copy()  # [128,3,2]

    def per_d(v):  # [D] -> [128, NC]
        return np.asarray(v, f32).reshape(NC, 128).T.copy()

    lng = np.stack([per_d(ln_g[l]) for l in range(NB)])   # [NB,128,3]
    lnb = np.stack([per_d(ln_b[l]) for l in range(NB)])
    nrmg = per_d(norm_g)
    nrmb = per_d(norm_b)
    p = np.arange(128, dtype=f32)[:, None]
    u = np.arange(MLW, dtype=f32)[None, :]
    win = (p - u + MLA).astype(np.float64)                # j - i window
    pbm_f = np.asarray(pbm, np.float64).reshape(NB)
    mask8 = np.empty((NB, 128, MLW), f32)
    for l in range(NB):
        sig = 1.0 / (1.0 + np.exp(-(win + pbm_f[l])))
        mask8[l] = np.where(win <= 0, sig, NEG_BIG).astype(f32)

    ew = np.asarray(expand_w, f32)                        # [NB, 1632, 384]
    # padded layout: [qh 0:48 | zeros 48:64 | kh 64:112 | lin 384 | pre 384]
    # 1/sqrt(QK) folded into the qh rows
    ewp = np.zeros((NB, 880, D), f32)
    ewp[:, 0:QK] = ew[:, 0:QK] * INV_SQRT_QK
    ewp[:, KOFF:KOFF + QK] = ew[:, QK:2 * QK]
    ewp[:, QKP:QKP + 384] = ew[:, 2 * QK:2 * QK + 384]
    ewp[:, 496:496 + 384] = ew[:, 2 * QK + EX:2 * QK + EX + 384]
    colsB = np.r_[2 * QK + 384:2 * QK + EX, 2 * QK + EX + 384:2 * QK + 2 * EX]
    # eA[l, c, p, fa] = ewp[l, fa, 128c+p]
    eA = np.transpose(ewp.reshape(NB, 880, NC, 128),
                      (0, 2, 3, 1)).astype(bf16)          # [NB,3,128,880]
    eB = np.transpose(ew[:, colsB, :].reshape(NB, 2 * D, NC, 128),
                      (0, 2, 3, 1)).astype(bf16)          # [NB,3,128,768]
    import ml_dtypes as _mld
    fp8 = _mld.float8_e4m3
    pw = np.asarray(project_w, f32)                       # [NB, 384, 768]
    # pW[l, e, p, d] = pw[l, d, 128e+p], then e paired: [NB, 3, 128, 2, D]
    pWe = np.transpose(pw.reshape(NB, D, NE, 128), (0, 2, 3, 1))
    pW = np.transpose(pWe.reshape(NB, NE // 2, 2, 128, D),
                      (0, 1, 3, 2, 4)).astype(fp8)        # [NB,3,128,2,384]
    oW = np.transpose(np.asarray(out_w, f32).reshape(VOCAB, NC, 128),
                      (1, 2, 0)).astype(bf16)             # [3, 128, 100]

    qf = np.asarray(q, f32)
    base = dict(fvec=fvec, bvec=bvec, ge=geT, lng=lng, lnb=lnb, nrmg=nrmg,
                nrmb=nrmb, mask8=mask8, eA=eA, eB=eB, pW=pW, oW=oW)
    maps = []
    for b in range(N_CORES):
        m = dict(base)
        m["qrow"] = qf[b].reshape(1, N).copy()
        maps.append(m)
    return maps


def kernel(q, group_emb, expand_w, project_w, ln_g, ln_b, pbm, norm_g, norm_b,
           out_w):
    _install_ntff_hook()
    from concourse.bass_utils import run_bass_kernel_spmd

    trivial_ln = bool(np.all(np.asarray(ln_g) == 1.0)
                      and np.all(np.asarray(ln_b) == 0.0))
    tr_norm = bool(np.all(np.asarray(norm_g) == 1.0)
                   and np.all(np.asarray(norm_b) == 0.0))
    key = ("nc", trivial_ln, tr_norm)
    if key not in _CACHE:
        _CACHE[key] = build_nc(trivial_ln=trivial_ln, trivial_norm=tr_norm)
    nc = _CACHE[key]

    in_maps = _prep_inputs(q, group_emb, expand_w, project_w, ln_g, ln_b, pbm,
                           norm_g, norm_b, out_w)
    import os
    trace = bool(int(os.environ.get("KERNEL_TRACE", "0")))
    res = run_bass_kernel_spmd(nc, in_maps, core_ids=list(range(N_CORES)),
                               trace=trace)
    _CACHE["last_result"] = res
    out = np.stack([res.results[b]["logits"] for b in range(N_CORES)])
    return out.astype(np.float32)


# revision 56
# speedup vs baseline: 1.1595x; 1.1595x over previous
"""Trainium2 Bass kernel for nn_Model_14336600834737 (dense_transformer).

Model (per batch element b, N=2048 tokens, D=384):
  x = LN(sincos_embed(q) + group_emb)
  8 blocks:
    h = LN_l(x);  z = h @ We^T  (f = [qh 48 | kh 48 | lin 768 | pre_gelu 768])
    geglu = lin * gelu(pre_gelu);  local, attn_v = geglu[:, :384], geglu[:, 384:]
    scores = qh kh^T / sqrt(48) + mask_l   (mask = sigmoid(j-i+pbm) causal, -inf else)
    att = softmax(scores) @ attn_v
    x = x + [local | att] @ P^T
  out = LN(x) @ out_w^T   -> [2048, 100]

Sharding: pure data parallel, 1 batch element per NeuronCore (8 cores).
On-chip layout: residual kept transposed xT [d, n] = 3 chunks of [128, 2048];
attention computed as S^T [j, i] tiles so softmax sums use ones-matmuls and
attention output lands directly as att^T [d, i] (no transposes anywhere).

v2: bf16 activations + weights (psum/residual stay fp32); all per-layer
tensors double-buffered so layer l+1 expand overlaps layer l attention;
ACT queue chained in emission order with only two table sets
(natural_log_exp for LN-rstd/mask/softmax, gelu set for geglu) so table
reloads drop from ~180 to ~18; qh/kh computed in one matmul.
"""

import math

import numpy as np

# ---- hardcoded problem shapes ----
B, N, D = 8, 2048, 384
NB, QK, EX, GS, VOCAB = 8, 48, 768, 2, 100
NT = 4            # n tiles per 2048
TW = 512          # tile width
NBLK = N // 128   # 16 row blocks
NC = 3            # d chunks (384/128)
NE = 6            # e chunks (768/128)
MLW = 1152        # mask window width
MLA = 384         # mask window offset: gwin[p,u] = p - u + MLA
INV_SQRT_QK = 1.0 / math.sqrt(QK)
QKP = 112         # padded qk rows: [qh 0:48 | 0 pad 48:64 | kh 64:112]
KOFF = 64         # kh partition offset (32-aligned)
QK4 = INV_SQRT_QK ** 0.5   # scale applied to both qh and kh
NEG_BIG = -1.0e30
N_CORES = 8

_CACHE = {}


def _install_ntff_hook():
    """Register the axon NTFF profile hook if missing (enables trace=True)."""
    import sys, types
    if "antenv.axon_hooks" in sys.modules:
        return
    mod = types.ModuleType("antenv.axon_hooks")
    _h = [None]
    mod.set_axon_ntff_profile_hook = lambda h: _h.__setitem__(0, h)
    mod.get_axon_ntff_profile_hook = lambda: _h[0]
    sys.modules["antenv.axon_hooks"] = mod
    try:
        import antenv
        antenv.axon_hooks = mod
        from trn_agent_boot.trn_boot import _ntff_profile_via_ctypes
        hook = _ntff_profile_via_ctypes("/opt/axon/libaxon_pjrt.so")
        if hook is not None:
            mod.set_axon_ntff_profile_hook(hook)
    except Exception:
        pass


def build_nc(trivial_ln=False, trivial_norm=False):
    import concourse.bacc as bacc
    import concourse.tile as tile
    from concourse.tile import add_dep_helper
    from concourse import mybir

    dt = mybir.dt
    AF = mybir.ActivationFunctionType
    ALU = mybir.AluOpType
    F32, F32R, BF16 = dt.float32, dt.float32r, dt.bfloat16
    FP8 = dt.float8e4
    DR = mybir.MatmulPerfMode.DoubleRow

    nc = bacc.Bacc("TRN2", target_bir_lowering=False, debug=False)

    # ---- DRAM I/O ----
    d_q = nc.dram_tensor("qrow", [1, N], F32, kind="ExternalInput")
    d_fvec = nc.dram_tensor("fvec", [128, NC], F32, kind="ExternalInput")
    d_bvec = nc.dram_tensor("bvec", [128, NC], F32, kind="ExternalInput")
    d_ge = nc.dram_tensor("ge", [128, NC, GS], F32, kind="ExternalInput")
    d_lng = nc.dram_tensor("lng", [NB, 128, NC], F32, kind="ExternalInput")
    d_lnb = nc.dram_tensor("lnb", [NB, 128, NC], F32, kind="ExternalInput")
    d_ng = nc.dram_tensor("nrmg", [128, NC], F32, kind="ExternalInput")
    d_nb = nc.dram_tensor("nrmb", [128, NC], F32, kind="ExternalInput")
    d_mask = nc.dram_tensor("mask8", [NB, 128, MLW], BF16, kind="ExternalInput")
    d_idI = nc.dram_tensor("idI", [128, 128], BF16, kind="ExternalInput")
    d_eA = nc.dram_tensor("eA", [NB, NC, 128, 880], BF16, kind="ExternalInput")
    d_eB = nc.dram_tensor("eB", [NB, NC, 128, 2 * D], BF16, kind="ExternalInput")
    d_pW = nc.dram_tensor("pW", [NB, NE // 2, 128, 2, D], FP8,
                          kind="ExternalInput")
    d_oW = nc.dram_tensor("oW", [NC, 128, VOCAB], BF16, kind="ExternalInput")
    d_out = nc.dram_tensor("logits", [N, VOCAB], F32, kind="ExternalOutput")

    def mm(out, lhsT, rhs, start, stop):
        """fp32 matmul in f32r fast mode."""
        nc.tensor.matmul(out, lhsT.bitcast(F32R), rhs.bitcast(F32R),
                         start=start, stop=stop)

    def mmb(out, lhsT, rhs, start, stop):
        """bf16 matmul."""
        nc.tensor.matmul(out, lhsT, rhs, start=start, stop=stop)

    with tile.TileContext(nc) as tc:
        import contextlib
        with contextlib.ExitStack() as ctx:
            ctx.enter_context(nc.allow_low_precision(
                reason="bf16/f32r matmuls within tolerance"))
            sb = ctx.enter_context(tc.tile_pool(name="sb", bufs=1))
            ps = ctx.enter_context(tc.tile_pool(name="ps", bufs=1, space="PSUM"))

            # ACT-queue chain: keep scalar-engine ops in emission order so the
            # scheduler can't interleave gelu-set and exp-set ops (each
            # alternation costs a ~1.3-2.7us table reload).
            _act_prev = [None]

            def _chain(i):
                if _act_prev[0] is not None:
                    add_dep_helper(i.ins, _act_prev[0].ins, sync=False,
                                   reason="ACT table-set phase order")
                _act_prev[0] = i
                return i

            def act(**kw):
                return _chain(nc.scalar.activation(**kw))

            def actc(out, in_):
                return _chain(nc.scalar.copy(out=out, in_=in_))

            # ---- constants into SBUF ----
            fvec = sb.tile([128, NC], F32, tag="fvec")
            nc.sync.dma_start(out=fvec, in_=d_fvec.ap())
            bvec = sb.tile([128, NC], F32, tag="bvec")
            nc.sync.dma_start(out=bvec, in_=d_bvec.ap())
            ge = sb.tile([128, NC, GS], F32, tag="ge")
            nc.sync.dma_start(out=ge, in_=d_ge.ap())
            lng = sb.tile([128, NB * NC], F32, tag="lng")
            lnb = sb.tile([128, NB * NC], F32, tag="lnb")
            for l in range(NB):
                nc.sync.dma_start(out=lng[:, NC * l:NC * l + NC], in_=d_lng.ap()[l])
                nc.sync.dma_start(out=lnb[:, NC * l:NC * l + NC], in_=d_lnb.ap()[l])
            nrmg = sb.tile([128, NC], F32, tag="nrmg")
            nc.sync.dma_start(out=nrmg, in_=d_ng.ap())
            nrmb = sb.tile([128, NC], F32, tag="nrmb")
            nc.sync.dma_start(out=nrmb, in_=d_nb.ap())
            oW = sb.tile([128, NC, VOCAB], BF16, tag="oW")
            for c in range(NC):
                nc.sync.dma_start(out=oW[:, c, :], in_=d_oW.ap()[c])
            qsb = sb.tile([128, N], F32, tag="qsb")
            nc.sync.dma_start(out=qsb, in_=d_q.ap().broadcast_to((128, N)))
            # identity weights for the mask-add matmul
            idI = sb.tile([128, 128], BF16, tag="idI")
            nc.sync.dma_start(out=idI, in_=d_idI.ap())

            # ones1 holds 1/D so the LN stats matmuls produce means directly
            ones1 = sb.tile([128, 1], F32R, tag="ones1")
            nc.vector.memset(ones1.bitcast(F32), 1.0 / D)
            ones2 = sb.tile([128, 2, 16], FP8, tag="ones2")
            nc.vector.memset(ones2, 1.0)
            onesr = sb.tile([1, 128], F32R, tag="onesr")
            nc.scalar.activation(out=onesr, in_=onesr.bitcast(F32),
                                 func=AF.Identity, scale=0.0, bias=1.0)
            eps = sb.tile([1, 1], F32, tag="eps")
            nc.vector.memset(eps, 1e-5)

            # ---- persistent activations ----
            xT = [sb.tile([128, N], F32, tag=f"x{c}", name=f"xT{c}") for c in range(NC)]

            def bcast_row(row_ap):
                """[1, TW] -> PSUM [128, TW] via K=1 ones matmul."""
                out = ps.tile([128, TW], F32, tag="att", bufs=3, name="bc")
                mm(out, onesr, row_ap, True, True)
                return out

            def layer_norm(dst_of, g_ap, b_ap, trivial_affine=False,
                           dst_dt=BF16):
                """LN over d (partition chunks) of xT -> dst tiles per (c, nt).

                Stats via ones-matmuls; rstd = exp(-0.5*ln(var+eps)) so the
                scalar engine never leaves the natural_log_exp table set;
                broadcasts via K=1 matmuls.
                """
                mus, sds = {}, {}
                for nt in range(NT):
                    s1 = ps.tile([1, TW], F32, tag="row", bufs=1, name="s1")
                    sqs = []
                    for c in range(NC):
                        xs = xT[c][:, TW * nt:TW * nt + TW]
                        sq = sb.tile([128, TW], F32, tag="sq", bufs=3, name="sq")
                        nc.gpsimd.tensor_mul(sq.bitcast(F32R), xs, xs)
                        sqs.append(sq)
                        mm(s1, ones1, xs, c == 0, c == NC - 1)
                    mu = sb.tile([1, TW], F32, tag="mu", bufs=NT, name="mu")
                    nc.vector.tensor_copy(mu.bitcast(F32R), s1)  # = mean
                    s2 = ps.tile([1, TW], F32, tag="row", bufs=1, name="s2")
                    for c in range(NC):
                        mm(s2, ones1, sqs[c], c == 0, c == NC - 1)
                    m2 = sb.tile([1, TW], F32, tag="m2", bufs=1, name="m2")
                    nc.vector.tensor_mul(m2, mu, mu)
                    msq = sb.tile([1, TW], F32, tag="msq", bufs=NT, name="msq")
                    nc.vector.tensor_sub(msq.bitcast(F32R), s2, m2)  # var
                    mus[nt], sds[nt] = mu, msq
                for nt in range(NT):
                    # sd = sqrt(var + eps); all 4 chained -> one table set
                    act(out=sds[nt].bitcast(F32R), in_=sds[nt], func=AF.Sqrt,
                        bias=eps)
                for nt in range(NT):
                    mu_b = bcast_row(mus[nt])
                    sd_b = bcast_row(sds[nt])
                    rstdb = sb.tile([128, TW], F32, tag="rstdb", bufs=2,
                                    name="rstdb")
                    nc.vector.reciprocal_approx_fast(out=rstdb, in_=sd_b)
                    for c in range(NC):
                        xs = xT[c][:, TW * nt:TW * nt + TW]
                        t1 = sb.tile([128, TW], F32, tag="t1", bufs=2, name="t1")
                        nc.vector.tensor_sub(t1, xs, mu_b)
                        if trivial_affine:
                            nc.vector.tensor_mul(dst_of(c, nt, dst_dt), t1,
                                                 rstdb)
                        else:
                            nc.vector.tensor_mul(t1, t1, rstdb)
                            nc.scalar.activation(out=dst_of(c, nt, dst_dt),
                                                 in_=t1, func=AF.Identity,
                                                 bias=b_ap[:, c:c + 1],
                                                 scale=g_ap[:, c:c + 1])

            # ---- embedding: xT[c][p, n] = sin(q[n]*fvec[p,c] + bvec[p,c]) ----
            for c in range(NC):
                act(out=xT[c].bitcast(F32R), in_=qsb, func=AF.Sin,
                    scale=fvec[:, c:c + 1], bias=bvec[:, c:c + 1])
                xr = xT[c].rearrange("p (a g) -> p a g", g=GS)
                geb = ge[:, c, :].unsqueeze(1).broadcast_to((128, N // GS, GS))
                nc.vector.tensor_add(xr.bitcast(F32R), xr, geb)

            # initial LN (in place on xT, fp32)
            def init_dst(c, nt, _dt):
                return xT[c][:, TW * nt:TW * nt + TW].bitcast(F32R)

            layer_norm(init_dst, nrmg, nrmb, trivial_affine=trivial_norm,
                       dst_dt=F32)

            # ---- transformer layers ----
            for l in range(NB):
                # layer weights (double-buffered across layers)
                eA = [sb.tile([128, 880], BF16, tag=f"eA{c}", bufs=2,
                              name=f"eA{c}_{l}") for c in range(NC)]
                eB = [sb.tile([128, 2 * D], BF16, tag=f"eB{c}", bufs=2,
                              name=f"eB{c}_{l}") for c in range(NC)]
                for c in range(NC):
                    nc.sync.dma_start(out=eA[c], in_=d_eA.ap()[l, c])
                    nc.sync.dma_start(out=eB[c], in_=d_eB.ap()[l, c])
                pW2 = [sb.tile([128, 2, D], FP8, tag=f"pW{k}", bufs=2,
                               name=f"pW{k}_{l}") for k in range(NE // 2)]
                for k in range(NE // 2):
                    nc.sync.dma_start(out=pW2[k], in_=d_pW.ap()[l, k])

                # per-layer additive mask window, precomputed on host:
                # sigmoid(j - i + pbm_l) where causal, -1e30 where not.
                Ml = sb.tile([128, MLW], BF16, tag="Ml", bufs=2, name=f"Ml_{l}")
                nc.sync.dma_start(out=Ml, in_=d_mask.ap()[l])

                # h = LN_l(x), then expand + geglu, per n-tile
                qh = sb.tile([QK, N], BF16, tag="qh", bufs=2, name=f"qh_{l}")
                kh = sb.tile([QK, N], BF16, tag="kh", bufs=2, name=f"kh_{l}")
                # projection rhs packed in fp8 e-chunk pairs for DoubleRow:
                # prj[k][p, o, n]; pairs = (loc0,loc1), (loc2,att0), (att1,att2)
                prj = [sb.tile([128, 2, N], FP8, tag=f"prj{k}", bufs=2,
                               name=f"prj{k}_{l}") for k in range(NE // 2)]
                # V packed in fp8 j-block pairs for DoubleRow P@V:
                # V2[jp][p, o, d] = V[j = 128*(2jp+o) + p][d]
                V2 = [sb.tile([128, 2, D], FP8, tag="v", bufs=NBLK,
                              name=f"v{jp}_{l}") for jp in range(NBLK // 2)]

                hcur = {}

                def make_h(c, nt, _dt, _l=l):
                    hcur[(c, nt)] = sb.tile([128, TW], _dt, tag="h", bufs=6,
                                            name=f"h{c}_{nt}_{_l}")
                    return hcur[(c, nt)]

                layer_norm(make_h, lng[:, NC * l:NC * l + NC],
                           lnb[:, NC * l:NC * l + NC], trivial_affine=trivial_ln)

                for nt in range(NT):
                    hnt = [hcur[(c, nt)] for c in range(NC)]
                    # qh/kh (transposed layout); 1/sqrt(QK) folded into the
                    # qh weight columns host-side; copies on the scalar engine
                    pq = ps.tile([QK, TW], F32, tag="s", bufs=4, name="pq")
                    for c in range(NC):
                        mmb(pq, eA[c][:, 0:QK], hnt[c], c == 0, c == NC - 1)
                    actc(qh[:, TW * nt:TW * nt + TW], pq)
                    pk = ps.tile([QK, TW], F32, tag="s", bufs=4, name="pk")
                    for c in range(NC):
                        mmb(pk, eA[c][:, KOFF:KOFF + QK], hnt[c],
                            c == 0, c == NC - 1)
                    actc(kh[:, TW * nt:TW * nt + TW], pk)
                    # local geglu, transposed layout [e-block, n]
                    for e in range(NC):
                        pl = ps.tile([128, TW], F32, tag="s", bufs=4, name="pl")
                        pg = ps.tile([128, TW], F32, tag="s", bufs=4, name="pg")
                        for c in range(NC):
                            mmb(pl, eA[c][:, QKP + 128 * e:QKP + 128 + 128 * e],
                                hnt[c], c == 0, c == NC - 1)
                        for c in range(NC):
                            mmb(pg, eA[c][:, 496 + 128 * e:624 + 128 * e],
                                hnt[c], c == 0, c == NC - 1)
                        gt = sb.tile([128, TW], F32, tag="gt", bufs=3, name="gt")
                        act(out=gt, in_=pg, func=AF.Gelu)
                        nc.vector.tensor_mul(
                            prj[e // 2][:, e % 2, TW * nt:TW * nt + TW], pl, gt)
                    # attn_v geglu, row layout V[j, d]
                    for k in range(NT):
                        nb = NT * nt + k
                        pvl = ps.tile([128, D], F32, tag="s", bufs=4, name="pvl")
                        pvg = ps.tile([128, D], F32, tag="s", bufs=4, name="pvg")
                        for c in range(NC):
                            mmb(pvl, hnt[c][:, 128 * k:128 * k + 128],
                                eB[c][:, 0:D], c == 0, c == NC - 1)
                        for c in range(NC):
                            mmb(pvg, hnt[c][:, 128 * k:128 * k + 128],
                                eB[c][:, D:2 * D], c == 0, c == NC - 1)
                        gt2 = sb.tile([128, D], F32, tag="gt", bufs=3, name="gt2")
                        act(out=gt2, in_=pvg, func=AF.Gelu)
                        nc.vector.tensor_mul(V2[nb // 2][:, nb % 2, :], pvl, gt2)

                # ---- attention, i-chunks of 512, S^T [j, i] tiles ----
                for ic in range(NT):
                    i0 = TW * ic
                    nj = NT * (ic + 1)
                    pa = [ps.tile([128, TW], F32, tag="att", bufs=3, name=f"pa{d}")
                          for d in range(NC)]
                    prow = ps.tile([1, TW], F32, tag="row", bufs=1, name="prow")
                    njp = nj // 2
                    for jp in range(njp):
                        es2 = sb.tile([128, 2, TW], FP8, tag="es", bufs=4,
                                      name="es2")
                        for o in range(2):
                            jb = 2 * jp + o
                            j0 = 128 * jb
                            # mask needed only near the diagonal: sigmoid
                            # underflows below offset -128
                            need = j0 - i0 >= -128
                            pss = ps.tile([128, TW], F32, tag="s", bufs=4,
                                          name="pss")
                            mmb(pss, kh[:, j0:j0 + 128], qh[:, i0:i0 + TW],
                                True, not need)
                            if need:
                                off = MLA - (j0 - i0)
                                mmb(pss, idI, Ml[:, off:off + TW],
                                    False, True)
                            act(out=es2[:, o, :], in_=pss, func=AF.Exp)
                        nc.tensor.matmul(prow, ones2[:, :, 0:1], es2,
                                         start=jp == 0, stop=jp == njp - 1,
                                         perf_mode=DR)
                        for d in range(NC):
                            nc.tensor.matmul(
                                pa[d], V2[jp][:, :, 128 * d:128 * d + 128],
                                es2, start=jp == 0, stop=jp == njp - 1,
                                perf_mode=DR)
                    rs = sb.tile([1, TW], F32, tag="rs", bufs=2, name="rs")
                    nc.vector.tensor_copy(rs.bitcast(F32R), prow)
                    s_b = ps.tile([128, TW], F32, tag="row", bufs=1, name="s_b")
                    mm(s_b, onesr, rs, True, True)
                    rsb = sb.tile([128, TW], F32, tag="rsb", bufs=2, name="rsb")
                    nc.vector.reciprocal_approx_fast(out=rsb, in_=s_b)
                    # att d-chunk d lands in prj pair (NC+d)//2, slot (NC+d)%2
                    for d in range(NC):
                        e = NC + d
                        nc.vector.tensor_mul(
                            prj[e // 2][:, e % 2, i0:i0 + TW], pa[d], rsb)
                    # projection for this n-tile + residual (fp8 DoubleRow)
                    for d2 in range(NC):
                        pp = ps.tile([128, TW], F32, tag="s", bufs=4, name="pp")
                        for k in range(NE // 2):
                            nc.tensor.matmul(
                                pp, pW2[k][:, :, 128 * d2:128 * d2 + 128],
                                prj[k][:, :, i0:i0 + TW],
                                start=k == 0, stop=k == NE // 2 - 1,
                                perf_mode=DR)
                        xs = xT[d2][:, i0:i0 + TW]
                        nc.vector.tensor_add(xs.bitcast(F32R), pp, xs)

            # ---- final LN + head ----
            hf = {}

            def make_hf(c, nt, _dt):
                hf[(c, nt)] = sb.tile([128, TW], _dt, tag="h", bufs=6,
                                      name=f"hf{c}_{nt}")
                return hf[(c, nt)]

            layer_norm(make_hf, nrmg, nrmb, trivial_affine=trivial_norm)
            for nt in range(NT):
                for k in range(NT):
                    nb = NT * nt + k
                    po = ps.tile([128, VOCAB], F32, tag="s", bufs=4, name="po")
                    for c in range(NC):
                        mmb(po, hf[(c, nt)][:, 128 * k:128 * k + 128],
                            oW[:, c, :], c == 0, c == NC - 1)
                    ot = sb.tile([128, VOCAB], F32, tag="ot", bufs=2, name="ot")
                    nc.scalar.copy(out=ot, in_=po)
                    nc.sync.dma_start(out=d_out.ap()[128 * nb:128 * nb + 128, :],
                                      in_=ot)

    nc.compile()
    return nc


def _prep_inputs(q, group_emb, expand_w, project_w, ln_g, ln_b, pbm, norm_g,
                 norm_b, out_w):
    """Host-side sharding + weight layout prep. Returns list of per-core maps."""
    import ml_dtypes
    f32 = np.float32
    bf16 = ml_dtypes.bfloat16
    freqs = np.exp(np.arange(0, D, 2, dtype=np.float64)
                   * (-math.log(10000.0) / D)).astype(f32)  # [192]
    fv = np.zeros((D,), f32)
    bv = np.zeros((D,), f32)
    fv[:192] = freqs
    fv[192:] = freqs
    bv[192:] = math.pi / 2.0
    fvec = fv.reshape(NC, 128).T.copy()           # [128, 3]
    bvec = bv.reshape(NC, 128).T.copy()

    ge = np.asarray(group_emb, f32).reshape(GS, D)      # [2, 384]
    geT = np.transpose(ge.reshape(GS, NC, 128), (2, 1, 0)).copy()  # [128,3,2]

    def per_d(v):  # [D] -> [128, NC]
        return np.asarray(v, f32).reshape(NC, 128).T.copy()

    lng = np.stack([per_d(ln_g[l]) for l in range(NB)])   # [NB,128,3]
    lnb = np.stack([per_d(ln_b[l]) for l in range(NB)])
    nrmg = per_d(norm_g)
    nrmb = per_d(norm_b)
    p = np.arange(128, dtype=f32)[:, None]
    u = np.arange(MLW, dtype=f32)[None, :]
    win = (p - u + MLA).astype(np.float64)                # j - i window
    pbm_f = np.asarray(pbm, np.float64).reshape(NB)
    mask8 = np.empty((NB, 128, MLW), np.float64)
    for l in range(NB):
        sig = 1.0 / (1.0 + np.exp(-(win + pbm_f[l])))
        mask8[l] = np.where(win <= 0, sig, NEG_BIG)
    mask8 = mask8.astype(bf16)

    ew = np.asarray(expand_w, f32)                        # [NB, 1632, 384]
    # padded layout: [qh 0:48 | zeros 48:64 | kh 64:112 | lin 384 | pre 384]
    # 1/sqrt(QK) folded into the qh rows
    ewp = np.zeros((NB, 880, D), f32)
    ewp[:, 0:QK] = ew[:, 0:QK] * INV_SQRT_QK
    ewp[:, KOFF:KOFF + QK] = ew[:, QK:2 * QK]
    ewp[:, QKP:QKP + 384] = ew[:, 2 * QK:2 * QK + 384]
    ewp[:, 496:496 + 384] = ew[:, 2 * QK + EX:2 * QK + EX + 384]
    colsB = np.r_[2 * QK + 384:2 * QK + EX, 2 * QK + EX + 384:2 * QK + 2 * EX]
    # eA[l, c, p, fa] = ewp[l, fa, 128c+p]
    eA = np.transpose(ewp.reshape(NB, 880, NC, 128),
                      (0, 2, 3, 1)).astype(bf16)          # [NB,3,128,880]
    eB = np.transpose(ew[:, colsB, :].reshape(NB, 2 * D, NC, 128),
                      (0, 2, 3, 1)).astype(bf16)          # [NB,3,128,768]
    import ml_dtypes as _mld
    fp8 = _mld.float8_e4m3
    pw = np.asarray(project_w, f32)                       # [NB, 384, 768]
    # pW[l, e, p, d] = pw[l, d, 128e+p], then e paired: [NB, 3, 128, 2, D]
    pWe = np.transpose(pw.reshape(NB, D, NE, 128), (0, 2, 3, 1))
    pW = np.transpose(pWe.reshape(NB, NE // 2, 2, 128, D),
                      (0, 1, 3, 2, 4)).astype(fp8)        # [NB,3,128,2,384]
    oW = np.transpose(np.asarray(out_w, f32).reshape(VOCAB, NC, 128),
                      (1, 2, 0)).astype(bf16)             # [3, 128, 100]

    qf = np.asarray(q, f32)
    idI = np.eye(128, dtype=f32).astype(bf16)
    base = dict(fvec=fvec, bvec=bvec, ge=geT, lng=lng, lnb=lnb, nrmg=nrmg,
                nrmb=nrmb, mask8=mask8, idI=idI, eA=eA, eB=eB, pW=pW, oW=oW)
    maps = []
    for b in range(N_CORES):
        m = dict(base)
        m["qrow"] = qf[b].reshape(1, N).copy()
        maps.append(m)
    return maps


def kernel(q, group_emb, expand_w, project_w, ln_g, ln_b, pbm, norm_g, norm_b,
           out_w):
    _install_ntff_hook()
    from concourse.bass_utils import run_bass_kernel_spmd

    trivial_ln = bool(np.all(np.asarray(ln_g) == 1.0)
                      and np.all(np.asarray(ln_b) == 0.0))
    tr_norm = bool(np.all(np.asarray(norm_g) == 1.0)
                   and np.all(np.asarray(norm_b) == 0.0))
    key = ("nc", trivial_ln, tr_norm)
    if key not in _CACHE:
        _CACHE[key] = build_nc(trivial_ln=trivial_ln, trivial_norm=tr_norm)
    nc = _CACHE[key]

    in_maps = _prep_inputs(q, group_emb, expand_w, project_w, ln_g, ln_b, pbm,
                           norm_g, norm_b, out_w)
    import os
    trace = bool(int(os.environ.get("KERNEL_TRACE", "0")))
    res = run_bass_kernel_spmd(nc, in_maps, core_ids=list(range(N_CORES)),
                               trace=trace)
    _CACHE["last_result"] = res
    out = np.stack([res.results[b]["logits"] for b in range(N_CORES)])
    return out.astype(np.float32)
